# revision 14
# baseline (speedup 1.0000x reference)
"""MoE (top-2 routed GluMLP) Trainium2 kernel, expert x DFF-half sharded
over 8 NeuronCores.

Contract: kernel(**inputs) takes the FULL unsharded inputs
  x  [2, 2048, 1024] f32
  Wr [8, 1024] f32           router
  Wg [8, 4096, 1024] f32     gate proj per expert
  Wu [8, 4096, 1024] f32     up proj per expert
  Wd [8, 1024, 4096] f32     down proj per expert
and returns the FULL output [2, 2048, 1024] f32.

Strategy:
  - Routing (softmax + top-2 + renormalize) on host with jax on CPU using
    the exact ops of the reference, so selected experts and combine weights
    match the reference bit-for-bit.
  - Work unit = (expert, DFF-half) shard: 16 shards over 8 cores, 2 per
    core. Experts are paired big-with-small by measured load, and the pair's
    two cores take complementary DFF halves of BOTH experts. Per-core work
    is ~(l_big + l_small)/2 of the dense per-expert capacity, which load-
    balances to ~1.03x the theoretical minimum without extra weight traffic
    or SBUF (each core still streams 16MB of Wg/Wu and holds 64KB/partition
    of Wd: half the DFF for two experts).
  - Each core computes unweighted partial GluMLP outputs for its two token
    slots (expert A tokens, then expert B tokens at a static offset):
        part[:, t] = Wd[:, half] @ (relu(Wg[half] @ x_t) * (Wu[half] @ x_t))
    Host sums the two half-partials per expert (exact fp32), applies the
    top-2 combine weights, and scatter-adds into the full output.
  - Matmul operands in fp16 (same 10-bit mantissa as TF32), fp32 PSUM
    accumulation; host converts operands so rounding is exact.
  - Phase B streams tokens against Wg/Wu tiles (f-major h layout). Phase C
    keeps the core's WdT half-pair resident in SBUF (prefetched behind
    phase B) and streams h as the moving operand against stationary Wd
    tiles: one uninterrupted matmul stream, output in [d, t] layout.

Env: MOE_MM_DT=f16 (default) | f32r (TF32, 2x DMA) | f32 (4x slower, exact).
"""

import math
import os
from contextlib import ExitStack

import numpy as np

import concourse.bass as bass
import concourse.tile as tile
from concourse import bacc, mybir
from concourse.bass_utils import run_bass_kernel_spmd

B, L, D, E, TOPK, DFF = 2, 2048, 1024, 8, 2, 4096
T = B * L
NCORES = 8
P = 128
NB = 512          # matmul moving-operand block (one PSUM bank of fp32 out)
DC = D // P       # 8 contraction chunks over D
DN = D // P       # 8 output chunks over D (phase C)
FM = DFF // P     # 32 chunks over DFF
HFM = FM // 2     # 16 chunks per DFF half (per-core share)

F32 = mybir.dt.float32
F32R = mybir.dt.float32r
F16 = mybir.dt.float16

# Set to True (e.g. from test.py) to run with NTFF tracing and print HW time.
PROFILE = False
TRACE_CORES = None  # e.g. list(range(8)) to profile every core
LAST_EXEC_NS = None
# Matmul dtype for the big GluMLP matmuls.
MM_DT = {"f32": F32, "f32r": F32R, "f16": F16}[os.environ.get("MOE_MM_DT", "f16")]

# dc consumption order matched to stripe arrival (sync: 3,6 behind wg0;
# scalar: 1,7 behind wu0; gpsimd: 2,5,0,4 -- SWDGE is fastest early) so the
# first fm's accumulation chain starts as soon as the first stripes land
# instead of waiting for the last.
DC_ORDER = [2, 5, 3, 1, 0, 6, 7, 4]


def _nblocks(tch):
    """Moving-dim blocks <=512, balanced: per-matmul cost is
    max(stream_cols/2.4GHz, ~100ns weight-load floor), so equal blocks beat
    512s-plus-tiny-tail (a tail below ~233 cols is pure LDW overhead)."""
    k = max(1, math.ceil(tch / NB))
    base, rem = divmod(tch, k)
    out, t = [], 0
    for i in range(k):
        nb = base + (1 if i < rem else 0)
        out.append((t, nb))
        t += nb
    return out


def _build_nc(W0: int, W1: int):
    """Build the single-core Bass program (SPMD: all cores run the same NEFF).

    Two token slots of static widths W0 (expert A) and W1 (expert B)."""
    nc = bacc.Bacc(
        "TRN2",
        target_bir_lowering=False,
        debug=False,
        enable_asserts=False,
        num_devices=NCORES,
    )
    mdt = MM_DT
    Tc = W0 + W1
    x_d = nc.dram_tensor("x", [P, DC, Tc], mdt, kind="ExternalInput").ap()
    # 32 weight tiles: slot0's 16 DFF-half tiles of expert A, then slot1's of B
    wg_d = nc.dram_tensor("wg", [FM, P, DC, P], mdt, kind="ExternalInput").ap()
    wu_d = nc.dram_tensor("wu", [FM, P, DC, P], mdt, kind="ExternalInput").ap()
    # WdT resident layout: [p(f_inner), slot*HFM + fc, dn, d_inner]
    wd_d = nc.dram_tensor("wd", [P, FM, DN, P], mdt, kind="ExternalInput").ap()
    # out in [dn, d_inner, t] layout; host transposes, sums half-partials,
    # and applies combine weights
    out_d = nc.dram_tensor("out", [DN, P, Tc], F32, kind="ExternalOutput").ap()

    with tile.TileContext(nc) as tc:
        with ExitStack() as ctx:
            _moe_body(ctx, tc, x_d, wg_d, wu_d, wd_d, out_d, W0, W1)
    nc.compile()
    return nc


def _moe_body(ctx, tc, x_d, wg_d, wu_d, wd_d, out_d, W0, W1):
    nc = tc.nc
    mdt = MM_DT
    slots = [(0, W0), (W0, W1)]
    Tc = W0 + W1

    xpool = ctx.enter_context(tc.tile_pool(name="xpool", bufs=1))
    hpool = ctx.enter_context(tc.tile_pool(name="hpool", bufs=1))
    # bufs=4: the 4 prefetched wg/wu pairs stay live at startup; fewer would
    # gate prefetch DMAs (and anything queued behind them) on fm0 finishing.
    wgupool = ctx.enter_context(tc.tile_pool(name="wgupool", bufs=4))
    wdpool = ctx.enter_context(tc.tile_pool(name="wdpool", bufs=1))
    gopool = ctx.enter_context(tc.tile_pool(name="gopool", bufs=2))
    # One shared PSUM pool: phase B (ps_g/ps_u) and phase C (per-dn out)
    # don't overlap in time, so both get all 8 banks.
    psP = ctx.enter_context(tc.tile_pool(name="psP", bufs=8, space="PSUM"))

    # Resident WdT: both DFF-half tile sets, streamed in behind phase B.
    wd_sb = wdpool.tile([P, FM, DN, P], mdt, tag="wd")
    x_sb = xpool.tile([P, DC, Tc], mdt, tag="x")
    h_all = hpool.tile([P, HFM, Tc], mdt, tag="h")

    # ---- Phase A: streamed loads, ordered for fastest matmul start ----
    # Queues: sync + scalar (HWDGE), gpsimd (SWDGE). DMA triggers cost
    # ~650ns of engine time each, so x moves as 8 whole dc-stripes per slot
    # (not per-block slices). First wg/wu tile leads its queue; slot-0
    # stripes next in DC_ORDER arrival positions; then the prefetch pairs
    # and slot-1 stripes behind them.
    stripe_q = {3: nc.sync, 6: nc.sync,
                1: nc.scalar, 7: nc.scalar,
                2: nc.gpsimd, 5: nc.gpsimd, 0: nc.gpsimd, 4: nc.gpsimd}
    pre = []
    wg_p = wgupool.tile([P, DC, P], mdt, tag="wg", name="wg_pre0")
    nc.sync.dma_start(out=wg_p, in_=wg_d[0])
    wu_p = wgupool.tile([P, DC, P], mdt, tag="wu", name="wu_pre0")
    nc.scalar.dma_start(out=wu_p, in_=wu_d[0])
    pre.append((wg_p, wu_p))
    for s, (off, W) in enumerate(slots):
        for dc in DC_ORDER:  # per-queue issue order = consumption order
            # slot-1 stripes all ride the gpsimd queue: they are not needed
            # until ~fm16, and keeping them off sync/scalar lets the JIT
            # wg/wu tiles land well ahead of consumption.
            q = stripe_q[dc] if s == 0 else nc.gpsimd
            q.dma_start(
                out=x_sb[:, dc, off : off + W], in_=x_d[:, dc, off : off + W]
            )
        if s == 0:
            for fm in range(1, 4):
                wg_p = wgupool.tile([P, DC, P], mdt, tag="wg", name=f"wg_pre{fm}")
                nc.sync.dma_start(out=wg_p, in_=wg_d[fm])
                wu_p = wgupool.tile([P, DC, P], mdt, tag="wu", name=f"wu_pre{fm}")
                nc.scalar.dma_start(out=wu_p, in_=wu_d[fm])
                pre.append((wg_p, wu_p))

    # ---- Phase B: h[f, t] = relu(g) * u, f-major layout, slot-major ----
    for s, (off, W) in enumerate(slots):
        blocks = _nblocks(W)
        for fl in range(HFM):
            fmg = s * HFM + fl
            if fmg < len(pre):
                wg_sb, wu_sb = pre[fmg]
            else:
                wg_sb = wgupool.tile([P, DC, P], mdt, tag="wg")
                nc.sync.dma_start(out=wg_sb, in_=wg_d[fmg])
                wu_sb = wgupool.tile([P, DC, P], mdt, tag="wu")
                nc.scalar.dma_start(out=wu_sb, in_=wu_d[fmg])
            if 8 <= fmg < 8 + HFM:
                # Wd prefetch: 16 slabs of 512KB, deferred past the latency-
                # critical early window (x/wg/wu own HBM until ~fm8) and done
                # by ~fm24 -- still ~70us ahead of phase C. Rotating queues.
                fc2 = fmg - 8
                eng = [nc.sync, nc.scalar, nc.gpsimd][fc2 % 3]
                eng.dma_start(
                    out=wd_sb[:, 2 * fc2 : 2 * fc2 + 2],
                    in_=wd_d[:, 2 * fc2 : 2 * fc2 + 2],
                )
            for nb0, nbl in blocks:
                ts = slice(off + nb0, off + nb0 + nbl)
                ps_g = psP.tile([P, NB], F32, tag="ps")
                ps_u = psP.tile([P, NB], F32, tag="ps")
                for i, dc in enumerate(DC_ORDER):
                    nc.tensor.matmul(
                        ps_g[:, :nbl],
                        lhsT=wg_sb[:, dc, :],
                        rhs=x_sb[:, dc, ts],
                        start=(i == 0),
                        stop=(i == DC - 1),
                    )
                for i, dc in enumerate(DC_ORDER):
                    nc.tensor.matmul(
                        ps_u[:, :nbl],
                        lhsT=wu_sb[:, dc, :],
                        rhs=x_sb[:, dc, ts],
                        start=(i == 0),
                        stop=(i == DC - 1),
                    )
                g_sb = gopool.tile([P, NB], F32, tag="g")
                nc.scalar.activation(
                    out=g_sb[:, :nbl],
                    in_=ps_g[:, :nbl],
                    func=mybir.ActivationFunctionType.Relu,
                )
                nc.vector.tensor_mul(h_all[:, fl, ts], g_sb[:, :nbl], ps_u[:, :nbl])

    # ---- Phase C: part[d, t] = WdT.T @ h, Wd stationary / h moving ----
    # Per (slot, token block, dn): one PSUM bank accumulates 16 back-to-back
    # matmuls; drains (plain copies) and output DMA rotate across engines /
    # queues and hide behind the next bank's matmuls.
    out_q = [nc.gpsimd, nc.sync, nc.scalar]
    qi = 0
    for s, (off, W) in enumerate(slots):
        for nb0, nbl in _nblocks(W):
            ts = slice(off + nb0, off + nb0 + nbl)
            for dn in range(DN):
                ps_o = psP.tile([P, NB], F32, tag="ps")
                for fl in range(HFM):
                    nc.tensor.matmul(
                        ps_o[:, :nbl],
                        lhsT=wd_sb[:, s * HFM + fl, dn, :],
                        rhs=h_all[:, fl, ts],
                        start=(fl == 0),
                        stop=(fl == HFM - 1),
                    )
                o_sb = gopool.tile([P, NB], F32, tag="o")
                if dn % 2 == 0:
                    nc.scalar.activation(
                        out=o_sb[:, :nbl],
                        in_=ps_o[:, :nbl],
                        func=mybir.ActivationFunctionType.Copy,
                    )
                else:
                    nc.vector.tensor_scalar_mul(o_sb[:, :nbl], ps_o[:, :nbl], 1.0)
                out_q[qi % 3].dma_start(
                    out=out_d[dn, :, off + nb0 : off + nb0 + nbl],
                    in_=o_sb[:, :nbl],
                )
                qi += 1


_NC_CACHE: dict = {}


def _get_nc(W0: int, W1: int):
    if (W0, W1) not in _NC_CACHE:
        _NC_CACHE[(W0, W1)] = _build_nc(W0, W1)
    return _NC_CACHE[(W0, W1)]


def _round_tf32(a):
    """Round-to-nearest-even fp32 -> TF32 (10-bit mantissa), as np.float32."""
    u = a.astype(np.float32).view(np.uint32).astype(np.uint64)
    lsb = (u >> 13) & 1
    r = (u + 0x0FFF + lsb) & 0xFFFFE000
    return r.astype(np.uint32).view(np.float32)


def _mm_round(a):
    """Convert a host array to the dtype/value the device matmuls consume."""
    if MM_DT is F32R:
        return _round_tf32(a)
    if MM_DT is F16:
        return np.ascontiguousarray(a, dtype=np.float16)
    return np.ascontiguousarray(a, dtype=np.float32)


def _route_host(x, Wr):
    """Reference-identical routing on host (jax on CPU, same ops as reference).

    Returns (k_ids [T, K] int, k_w [T, K] f32).
    """
    import jax
    import jax.numpy as jnp

    cpu = jax.devices("cpu")[0]
    with jax.default_device(cpu):
        xt = jnp.asarray(x.reshape(T, D))
        logits = jnp.einsum("td,ed->te", xt, jnp.asarray(Wr))
        scores = jax.nn.softmax(logits, axis=-1)
        k_scores, k_ids = jax.lax.top_k(scores, TOPK)
        eps = jnp.finfo(x.dtype).eps
        k_w = k_scores / (k_scores.sum(axis=-1, keepdims=True) + eps)
        return np.asarray(k_ids), np.asarray(k_w)


def _prep_weights(Wg, Wu, Wd):
    """Per-expert weight tensors in device layouts (contiguous, rounded)."""
    wg_r, wu_r, wd_r = [], [], []
    for e in range(len(Wg)):
        # Wg[e]: [DFF, D]; device wants [fm, p(d_inner), dc, f_inner]
        wgt = Wg[e].T.reshape(DC, P, FM, P).transpose(2, 1, 0, 3)
        wut = Wu[e].T.reshape(DC, P, FM, P).transpose(2, 1, 0, 3)
        # Wd[e]: [D, DFF]; device wants [p(f_inner), fc, dn, d_inner]
        wdt = Wd[e].reshape(DN, P, FM, P).transpose(3, 2, 0, 1)
        wg_r.append(_mm_round(np.ascontiguousarray(wgt, dtype=np.float32)))
        wu_r.append(_mm_round(np.ascontiguousarray(wut, dtype=np.float32)))
        wd_r.append(_mm_round(np.ascontiguousarray(wdt, dtype=np.float32)))
    return wg_r, wu_r, wd_r


def kernel(x, Wr, Wg, Wu, Wd):
    global LAST_EXEC_NS
    x = np.asarray(x, dtype=np.float32)
    Wr = np.asarray(Wr, dtype=np.float32)
    Wg = np.asarray(Wg, dtype=np.float32)
    Wu = np.asarray(Wu, dtype=np.float32)
    Wd = np.asarray(Wd, dtype=np.float32)

    k_ids, k_w = _route_host(x, Wr)
    xt = x.reshape(T, D)

    # Gather per-expert token lists (each token appears once per selected expert).
    idx_lists, w_lists = [], []
    for e in range(E):
        tmask = k_ids == e                       # [T, K]
        tok = np.nonzero(tmask.any(axis=1))[0]   # unique tokens routed to e
        wvals = (k_w * tmask).sum(axis=1)[tok].astype(np.float32)
        idx_lists.append(tok)
        w_lists.append(wvals)

    loads = np.array([len(t) for t in idx_lists])
    # Pair big with small by load; the pair's two cores take complementary
    # DFF halves of both experts. Slot widths are the max load over pairs,
    # rounded to 8 tokens (16B-aligned fp16 DMA runs).
    order = np.argsort(-loads, kind="stable")
    pairs = [(int(order[p]), int(order[E - 1 - p])) for p in range(E // 2)]
    W0 = max(P, ((int(max(loads[a] for a, _ in pairs)) + 7) // 8) * 8)
    W1 = max(P, ((int(max(loads[b] for _, b in pairs)) + 7) // 8) * 8)
    Tc = W0 + W1
    # SBUF budget: h (16*Tc*2) + x (8*Tc*2) + resident Wd (64KB) + pools.
    assert Tc <= 2400, f"unexpectedly imbalanced routing (Tc={Tc})"

    wg_r, wu_r, wd_r = _prep_weights(Wg, Wu, Wd)

    in_maps = []
    for a, b in pairs:
        xg = np.zeros((Tc, D), dtype=np.float32)
        xg[: loads[a]] = xt[idx_lists[a]]
        xg[W0 : W0 + loads[b]] = xt[idx_lists[b]]
        # device layout [p(d_inner), dc, t]
        xg_r = _mm_round(
            np.ascontiguousarray(
                xg.T.reshape(DC, P, Tc).transpose(1, 0, 2), dtype=np.float32
            )
        )
        for h in range(2):
            sl = slice(h * HFM, (h + 1) * HFM)
            in_maps.append(
                {
                    "x": xg_r,
                    "wg": np.ascontiguousarray(
                        np.concatenate([wg_r[a][sl], wg_r[b][sl]], axis=0)
                    ),
                    "wu": np.ascontiguousarray(
                        np.concatenate([wu_r[a][sl], wu_r[b][sl]], axis=0)
                    ),
                    "wd": np.ascontiguousarray(
                        np.concatenate([wd_r[a][:, sl], wd_r[b][:, sl]], axis=1)
                    ),
                }
            )

    nc = _get_nc(W0, W1)
    core_ids = list(range(NCORES))
    if PROFILE:
        res = _run_profiled(nc, in_maps, core_ids)
        LAST_EXEC_NS = res.exec_time_ns
        results = res.results
    else:
        results = run_bass_kernel_spmd(nc, in_maps, core_ids).results

    out = np.zeros((T, D), dtype=np.float32)
    for p, (a, b) in enumerate(pairs):
        # device out: [dn, d_inner, t]; the two half-cores' partials sum to
        # the full GluMLP output (exact fp32 adds).
        oe = (
            results[2 * p]["out"].reshape(D, Tc)
            + results[2 * p + 1]["out"].reshape(D, Tc)
        )
        out[idx_lists[a]] += w_lists[a][:, None] * oe[:, : loads[a]].T
        out[idx_lists[b]] += w_lists[b][:, None] * oe[:, W0 : W0 + loads[b]].T
    return out.reshape(B, L, D)


def _run_profiled(nc, in_maps, core_ids):
    """run_bass_kernel_spmd with trace=True, providing the NTFF hook that the
    agent image's antenv stub lacks, and skipping the artifact upload."""
    import sys
    import tempfile
    import types

    import concourse.bass_utils as bu

    if "antenv.axon_hooks" not in sys.modules:
        from trn_agent_boot.trn_boot import _ntff_profile_via_ctypes

        hook = _ntff_profile_via_ctypes("/opt/axon/libaxon_pjrt.so")
        mod = types.ModuleType("antenv.axon_hooks")
        mod.get_axon_ntff_profile_hook = lambda: hook
        mod.set_axon_ntff_profile_hook = lambda h: None
        sys.modules["antenv.axon_hooks"] = mod

    orig_upload = bu.upload_artifacts
    bu.upload_artifacts = lambda tmpdir: ""
    try:
        return run_bass_kernel_spmd(
            nc,
            in_maps,
            core_ids,
            trace=True,
            trace_cores=TRACE_CORES,
            tmpdir=tempfile.mkdtemp(prefix="moe_ntff_"),
        )
    finally:
        bu.upload_artifacts = orig_upload


if __name__ == "__main__":
    # smoke test with random data (no reference comparison)
    rng = np.random.default_rng(0)
    ins = {
        "x": rng.standard_normal((B, L, D), dtype=np.float32),
        "Wr": (rng.standard_normal((E, D)) * 0.02).astype(np.float32),
        "Wg": (rng.standard_normal((E, DFF, D)) * 0.02).astype(np.float32),
        "Wu": (rng.standard_normal((E, DFF, D)) * 0.02).astype(np.float32),
        "Wd": (rng.standard_normal((E, D, DFF)) * 0.02).astype(np.float32),
    }
    out = kernel(**ins)
    print("out", out.shape, out.dtype, float(np.abs(out).max()))
